# revision 7
# baseline (speedup 1.0000x reference)
"""BiMultiHeadAttention kernel for 8 Trainium2 NeuronCores.

Sharding: 8 cores = 4 batches x 2 halves of the image-token axis (8192 rows
each). Each core computes all 8 heads for its slice:
  - projections (q, val_v) from an on-chip transposed v slice,
  - S = q.k^T in both orientations (softmax over text needs [ti,tl];
    the text->image numerator needs [tl,ti]),
  - exp on ScalarE with fused free-axis sums (the reference's max
    subtraction / clamp are softmax-invariant at these value ranges),
  - row-normalized attention @ values and the out_v output projection.
The text-side softmax denominators (Z_l) and numerators (N_l) are partial
sums over each core's ti half; the host merges them and applies the tiny
out_l projection. All matmuls run as float32r (~1e-4 relative error,
4x the fp32 matmul rate).
"""
import sys
sys.path.insert(0, "/opt/trn_rl_repo")
import numpy as np

V_DIM, L_DIM, EMBED, HEADS = 256, 256, 1024, 8
HEAD_DIM = EMBED // HEADS           # 128
SCALE = HEAD_DIM ** -0.5
CLAMP = 50000.0
BSZ, NIMG, NTEXT = 4, 16384, 256
NCORES = 8
HALF = NIMG // 2                    # 8192 rows per core
TI_CHUNK = 512
NCHUNK = HALF // TI_CHUNK           # 16
NSUB = TI_CHUNK // 128              # 4
NTLC = NTEXT // 128                 # 2
NCC = V_DIM // 128                  # 2

_prog_cache = {}


def _build_program():
    from concourse import bacc, mybir, tile
    from concourse.alu_op_type import AluOpType

    F32 = mybir.dt.float32
    F32R = mybir.dt.float32r
    EXP = mybir.ActivationFunctionType.Exp

    nc = bacc.Bacc("TRN2", target_bir_lowering=False)

    v_d = nc.declare_dram_parameter("v", [HALF, V_DIM], F32, isOutput=False)
    l_d = nc.declare_dram_parameter("l", [NTEXT, L_DIM], F32, isOutput=False)
    wq_d = nc.declare_dram_parameter("wq", [V_DIM, EMBED], F32R, isOutput=False)
    wl_d = nc.declare_dram_parameter("wl", [L_DIM, EMBED], F32R, isOutput=False)
    wvv_d = nc.declare_dram_parameter("wvv", [V_DIM, EMBED], F32R, isOutput=False)
    wvl_d = nc.declare_dram_parameter("wvl", [L_DIM, EMBED], F32R, isOutput=False)
    wo_d = nc.declare_dram_parameter("wo", [EMBED, V_DIM], F32R, isOutput=False)
    bq_d = nc.declare_dram_parameter("bq", [128, HEADS], F32, isOutput=False)
    bk_d = nc.declare_dram_parameter("bk", [128, HEADS], F32, isOutput=False)
    bvv_d = nc.declare_dram_parameter("bvv", [128, EMBED], F32, isOutput=False)
    bvl_d = nc.declare_dram_parameter("bvl", [128, EMBED], F32, isOutput=False)
    id_d = nc.declare_dram_parameter("ident", [128, 128], F32, isOutput=False)
    ones_d = nc.declare_dram_parameter("ones", [128, 128], F32R, isOutput=False)

    outv_d = nc.declare_dram_parameter("out_v", [HALF, V_DIM], F32, isOutput=True)
    nl_d = nc.declare_dram_parameter("nl", [128, HEADS * NTEXT], F32, isOutput=True)
    zl_d = nc.declare_dram_parameter("zl", [128, HEADS * NTLC], F32, isOutput=True)

    with tile.TileContext(nc) as tc:
        with (
            tc.tile_pool(name="constp", bufs=1) as constp,
            tc.tile_pool(name="stage", bufs=3) as stage,
            tc.tile_pool(name="work", bufs=1) as work,
            tc.tile_pool(name="sb", bufs=2) as sb,
            tc.tile_pool(name="mm", bufs=4, space="PSUM") as mmp,
            tc.tile_pool(name="accps", bufs=1, space="PSUM") as accps,
        ):
            def mk(pool, shape, dt, tag, bufs=None):
                return pool.tile(shape, dt, tag=tag, name=tag, bufs=bufs)

            ident = mk(constp, [128, 128], F32, "ident")
            nc.sync.dma_start(ident[:], id_d[:])
            ones_t = mk(constp, [128, 128], F32R, "ones_t")
            nc.sync.dma_start(ones_t[:], ones_d[:])

            def load_r(dram, rows, cols, tag):
                tiles = []
                for i in range(rows // 128):
                    t = mk(constp, [128, cols], F32R, f"{tag}{i}")
                    nc.sync.dma_start(t[:], dram[i * 128:(i + 1) * 128, :])
                    tiles.append(t)
                return tiles

            wq = load_r(wq_d, V_DIM, EMBED, "wq")     # [cc][128, 1024]
            wl = load_r(wl_d, L_DIM, EMBED, "wl")
            wvv = load_r(wvv_d, V_DIM, EMBED, "wvv")
            wvl = load_r(wvl_d, L_DIM, EMBED, "wvl")
            wo = load_r(wo_d, EMBED, V_DIM, "wo")     # [h][128, 256]
            bq = mk(constp, [128, HEADS], F32, "bq")
            nc.sync.dma_start(bq[:], bq_d[:])
            bk = mk(constp, [128, HEADS], F32, "bk")
            nc.sync.dma_start(bk[:], bk_d[:])
            bvv = mk(constp, [128, EMBED], F32, "bvv")
            nc.sync.dma_start(bvv[:], bvv_d[:])
            bvl = mk(constp, [128, EMBED], F32, "bvl")
            nc.sync.dma_start(bvl[:], bvl_d[:])

            # ---------------- phase 0: text-side tensors ----------------
            l_s = []
            for t in range(NTLC):
                ls = mk(stage, [128, L_DIM], F32, f"ls{t}")
                nc.sync.dma_start(ls[:], l_d[t * 128:(t + 1) * 128, :])
                l_s.append(ls)
            lt = [mk(work, [128, NTEXT], F32R, f"lt{cc}") for cc in range(NCC)]
            for cc in range(NCC):
                pt = mk(mmp, [128, 512], F32, "mm")
                for t in range(NTLC):
                    nc.tensor.transpose(pt[:, t * 128:(t + 1) * 128],
                                        l_s[t][:, cc * 128:(cc + 1) * 128], ident[:])
                nc.vector.tensor_copy(lt[cc][:], pt[:, :NTEXT])

            kt = [mk(work, [128, NTEXT], F32R, f"kt{h}") for h in range(HEADS)]
            for h in range(HEADS):
                pk = mk(mmp, [128, 512], F32, "mm")
                for cc in range(NCC):
                    nc.tensor.matmul(pk[:, :NTEXT], wl[cc][:, h * 128:(h + 1) * 128],
                                     lt[cc][:], start=(cc == 0), stop=(cc == NCC - 1))
                nc.vector.tensor_scalar_add(kt[h][:], pk[:, :NTEXT], bk[:, h:h + 1])

            vall = [mk(work, [128, EMBED], F32R, f"vall{t}") for t in range(NTLC)]
            for t in range(NTLC):
                for eg in range(2):
                    pv = mk(mmp, [128, 512], F32, "mm")
                    for cc in range(NCC):
                        nc.tensor.matmul(pv[:], lt[cc][:, t * 128:(t + 1) * 128],
                                         wvl[cc][:, eg * 512:(eg + 1) * 512],
                                         start=(cc == 0), stop=(cc == NCC - 1))
                    nc.vector.tensor_tensor(vall[t][:, eg * 512:(eg + 1) * 512], pv[:],
                                            bvl[:, eg * 512:(eg + 1) * 512],
                                            op=AluOpType.add)

            nl_acc = mk(work, [128, HEADS * NTEXT], F32, "nl_acc")
            nc.vector.memset(nl_acc[:], 0.0)
            zl_acc = mk(work, [128, HEADS * NTLC], F32, "zl_acc")
            nc.vector.memset(zl_acc[:], 0.0)

            fin_ps = [mk(accps, [128, 512], F32, f"fin{i}") for i in range(2)]

            # ---------------- phase 1: chunk loop ----------------
            for ci in range(NCHUNK):
                ti0 = ci * TI_CHUNK

                vs = []
                for s in range(NSUB):
                    vtile = mk(stage, [128, V_DIM], F32, f"vs{s}")
                    nc.sync.dma_start(
                        vtile[:], v_d[ti0 + s * 128: ti0 + (s + 1) * 128, :])
                    vs.append(vtile)
                vt = [mk(sb, [128, TI_CHUNK], F32R, f"vt{cc}") for cc in range(NCC)]
                for cc in range(NCC):
                    ptv = mk(mmp, [128, 512], F32, "mm")
                    for s in range(NSUB):
                        nc.tensor.transpose(ptv[:, s * 128:(s + 1) * 128],
                                            vs[s][:, cc * 128:(cc + 1) * 128],
                                            ident[:])
                    nc.vector.tensor_copy(vt[cc][:], ptv[:])

                qt = [mk(sb, [128, TI_CHUNK], F32R, f"qt{h}", bufs=1)
                      for h in range(HEADS)]
                for h in range(HEADS):
                    pq = mk(mmp, [128, 512], F32, "mm")
                    for cc in range(NCC):
                        nc.tensor.matmul(pq[:], wq[cc][:, h * 128:(h + 1) * 128],
                                         vt[cc][:], start=(cc == 0),
                                         stop=(cc == NCC - 1))
                    nc.vector.tensor_scalar_add(qt[h][:], pq[:], bq[:, h:h + 1])

                valv = [mk(sb, [128, EMBED], F32R, f"valv{s}", bufs=1)
                        for s in range(NSUB)]
                for s in range(NSUB):
                    for eg in range(2):
                        pv = mk(mmp, [128, 512], F32, "mm")
                        for cc in range(NCC):
                            nc.tensor.matmul(pv[:], vt[cc][:, s * 128:(s + 1) * 128],
                                             wvv[cc][:, eg * 512:(eg + 1) * 512],
                                             start=(cc == 0), stop=(cc == NCC - 1))
                        nc.vector.tensor_tensor(valv[s][:, eg * 512:(eg + 1) * 512],
                                                pv[:], bvv[:, eg * 512:(eg + 1) * 512],
                                                op=AluOpType.add)

                zl_chunk = mk(sb, [128, HEADS * NTLC], F32, "zl_chunk")

                for h in range(HEADS):
                    # S_T [tl_chunk, ti] -> E_T with fused Z_l partial
                    et = []
                    for t in range(NTLC):
                        pst = mk(mmp, [128, 512], F32, "mm")
                        nc.tensor.matmul(pst[:], kt[h][:, t * 128:(t + 1) * 128],
                                         qt[h][:], start=True, stop=True)
                        e = mk(sb, [128, TI_CHUNK], F32R, f"et{t}")
                        nc.scalar.activation(
                            e[:], pst[:], EXP,
                            accum_out=zl_chunk[:, h * NTLC + t: h * NTLC + t + 1])
                        et.append(e)
                    # S natural [ti_sub, tl] -> E natural (two ti_subs per bank)
                    en_pk = []
                    for half in range(2):
                        psn = mk(mmp, [128, 512], F32, "mm")
                        for si in range(2):
                            s = half * 2 + si
                            nc.tensor.matmul(psn[:, si * 256:(si + 1) * 256],
                                             qt[h][:, s * 128:(s + 1) * 128],
                                             kt[h][:], start=True, stop=True)
                        e = mk(sb, [128, 512], F32R, f"en{half}")
                        nc.scalar.activation(e[:], psn[:], EXP)
                        en_pk.append(e)
                    # Z_v broadcast via all-ones stationary matmul, then 1/Z
                    pz = mk(mmp, [128, 512], F32, "mm")
                    for t in range(NTLC):
                        nc.tensor.matmul(pz[:], ones_t[:], et[t][:],
                                         start=(t == 0), stop=(t == NTLC - 1))
                    zr = mk(sb, [128, 512], F32, "zr")
                    nc.vector.reciprocal(zr[:], pz[:])
                    # out_v head slice, normalized
                    pov = mk(mmp, [128, 512], F32, "mm")
                    for t in range(NTLC):
                        nc.tensor.matmul(pov[:], vall[t][:, h * 128:(h + 1) * 128],
                                         et[t][:], start=(t == 0),
                                         stop=(t == NTLC - 1))
                    ov = mk(sb, [128, TI_CHUNK], F32R, f"ov{h}", bufs=1)
                    nc.vector.tensor_tensor(ov[:], pov[:], zr[:], op=AluOpType.mult)
                    # N_l partial for this head
                    pnl = mk(accps, [128, NTEXT], F32, "pnl", bufs=2)
                    for s in range(NSUB):
                        nc.tensor.matmul(
                            pnl[:],
                            valv[s][:, h * 128:(h + 1) * 128],
                            en_pk[s // 2][:, (s % 2) * 256:(s % 2 + 1) * 256],
                            start=(s == 0), stop=(s == NSUB - 1))
                    nc.vector.tensor_tensor(nl_acc[:, h * NTEXT:(h + 1) * NTEXT],
                                            nl_acc[:, h * NTEXT:(h + 1) * NTEXT],
                                            pnl[:], op=AluOpType.add)
                    # out_v final projection, accumulated over heads
                    for s in range(NSUB):
                        nc.tensor.matmul(
                            fin_ps[s // 2][:, (s % 2) * 256:(s % 2 + 1) * 256],
                            ov[:, s * 128:(s + 1) * 128], wo[h][:],
                            start=(h == 0 and s % 2 == 0), stop=(h == HEADS - 1),
                            skip_group_check=True)

                nc.vector.tensor_tensor(zl_acc[:], zl_acc[:], zl_chunk[:],
                                        op=AluOpType.add)
                for s in range(NSUB):
                    fo = mk(stage, [128, V_DIM], F32, "fo")
                    nc.scalar.copy(fo[:], fin_ps[s // 2][:, (s % 2) * 256:(s % 2 + 1) * 256])
                    nc.sync.dma_start(
                        outv_d[ti0 + s * 128: ti0 + (s + 1) * 128, :], fo[:])

            nc.sync.dma_start(nl_d[:], nl_acc[:])
            nc.sync.dma_start(zl_d[:], zl_acc[:])

    nc.compile()
    return nc


def _make_runner(nc):
    """Build the sharded jitted executable once (mirrors
    bass2jax.run_bass_via_pjrt's multi-core path) so repeated calls skip
    retracing and the NEFF compile."""
    import jax
    from jax.sharding import Mesh, PartitionSpec
    from jax.experimental.shard_map import shard_map
    from concourse import bass2jax, mybir

    bass2jax.install_neuronx_cc_hook()
    partition_name = (nc.partition_id_tensor.name
                      if nc.partition_id_tensor else None)
    param_names, out_names, out_avals, zero_outs = [], [], [], []
    for alloc in nc.m.functions[0].allocations:
        if not isinstance(alloc, mybir.MemoryLocationSet):
            continue
        name = alloc.memorylocations[0].name
        if alloc.kind == "ExternalInput":
            if name != partition_name:
                param_names.append(name)
        elif alloc.kind == "ExternalOutput":
            out_names.append(name)
            shape = tuple(alloc.tensor_shape)
            dtype = mybir.dt.np(alloc.dtype)
            out_avals.append(jax.core.ShapedArray(shape, dtype))
            zero_outs.append(np.zeros(shape, dtype))
    n_params = len(param_names)
    n_outs = len(out_avals)
    in_names = param_names + out_names
    if partition_name is not None:
        in_names = in_names + [partition_name]

    def _body(*args):
        operands = list(args)
        if partition_name is not None:
            operands.append(bass2jax.partition_id_tensor())
        outs = bass2jax._bass_exec_p.bind(
            *operands,
            out_avals=tuple(out_avals),
            in_names=tuple(in_names),
            out_names=tuple(out_names),
            lowering_input_output_aliases=(),
            sim_require_finite=True,
            sim_require_nnan=True,
            nc=nc,
        )
        return tuple(outs)

    devices = jax.devices()[:NCORES]
    mesh = Mesh(np.asarray(devices), ("core",))
    in_specs = (PartitionSpec("core"),) * (n_params + n_outs)
    out_specs = (PartitionSpec("core"),) * n_outs
    sharded = jax.jit(
        shard_map(_body, mesh=mesh, in_specs=in_specs, out_specs=out_specs,
                  check_rep=False),
        donate_argnums=tuple(range(n_params, n_params + n_outs)),
        keep_unused=True,
    )
    return sharded, param_names, out_names, out_avals, zero_outs


def _run_spmd(nc, in_maps, timed_iters=0):
    import time as _time
    import jax

    if "runner" not in _prog_cache:
        _prog_cache["runner"] = _make_runner(nc)
    sharded, in_names, out_names, out_avals, zero_outs = _prog_cache["runner"]

    concat_in = [
        np.concatenate([np.asarray(m[name]) for m in in_maps], axis=0)
        for name in in_names
    ]
    concat_zeros = [
        np.zeros((NCORES * z.shape[0], *z.shape[1:]), z.dtype) for z in zero_outs
    ]
    out_arrs = sharded(*concat_in, *concat_zeros)
    out_arrs = [np.asarray(o) for o in out_arrs]

    if timed_iters:
        dev_in = [jax.device_put(x) for x in concat_in]
        for x in dev_in:
            x.block_until_ready()
        times = []
        for _ in range(timed_iters):
            zs = [np.zeros((NCORES * z.shape[0], *z.shape[1:]), z.dtype)
                  for z in zero_outs]
            dzs = [jax.device_put(z) for z in zs]
            for z in dzs:
                z.block_until_ready()
            t0 = _time.perf_counter()
            outs = sharded(*dev_in, *dzs)
            for o in outs:
                o.block_until_ready()
            times.append(_time.perf_counter() - t0)
        _prog_cache["exec_times_s"] = times

    results = []
    for c in range(NCORES):
        results.append({
            name: out_arrs[i].reshape(NCORES, *out_avals[i].shape)[c]
            for i, name in enumerate(out_names)
        })
    return results


def _numpy_fallback(v, l, attention_mask_v, attention_mask_l,
                    v_proj_w, v_proj_b, l_proj_w, l_proj_b,
                    values_v_w, values_v_b, values_l_w, values_l_b,
                    out_v_w, out_v_b, out_l_w, out_l_b):
    bsz, tgt_len, _ = v.shape
    src_len = l.shape[1]

    def heads(x, slen):
        return x.reshape(bsz, slen, HEADS, HEAD_DIM).transpose(0, 2, 1, 3)

    q = (v @ v_proj_w.T + v_proj_b) * SCALE
    k = l @ l_proj_w.T + l_proj_b
    val_v = heads(v @ values_v_w.T + values_v_b, tgt_len)
    val_l = heads(l @ values_l_w.T + values_l_b, src_len)
    qh = heads(q, tgt_len)
    kh = heads(k, src_len)
    attn = np.einsum('bhid,bhjd->bhij', qh, kh)
    attn = attn - attn.max()
    attn = np.clip(attn, -CLAMP, CLAMP)
    attn_t = attn.transpose(0, 1, 3, 2)
    attn_l = attn_t - attn_t.max(axis=-1, keepdims=True)
    attn_l = np.clip(attn_l, -CLAMP, CLAMP)
    attn_l = np.where(attention_mask_v[:, None, None, :], -np.inf, attn_l)
    m = attn_l.max(axis=-1, keepdims=True)
    e = np.exp(attn_l - m)
    attn_l = e / e.sum(axis=-1, keepdims=True)
    attn_v = np.where(attention_mask_l[:, None, None, :], -np.inf, attn)
    m = attn_v.max(axis=-1, keepdims=True)
    e = np.exp(attn_v - m)
    attn_v = e / e.sum(axis=-1, keepdims=True)
    out_v = np.einsum('bhij,bhjd->bhid', attn_v, val_l)
    out_l = np.einsum('bhij,bhjd->bhid', attn_l, val_v)
    out_v = out_v.transpose(0, 2, 1, 3).reshape(bsz, tgt_len, EMBED)
    out_l = out_l.transpose(0, 2, 1, 3).reshape(bsz, src_len, EMBED)
    out_v = out_v @ out_v_w.T + out_v_b
    out_l = out_l @ out_l_w.T + out_l_b
    return out_v.astype(np.float32), out_l.astype(np.float32)


def kernel(v, l, attention_mask_v, attention_mask_l,
           v_proj_w, v_proj_b, l_proj_w, l_proj_b,
           values_v_w, values_v_b, values_l_w, values_l_b,
           out_v_w, out_v_b, out_l_w, out_l_b):
    import os
    v = np.asarray(v, dtype=np.float32)
    l = np.asarray(l, dtype=np.float32)
    args = dict(
        attention_mask_v=np.asarray(attention_mask_v),
        attention_mask_l=np.asarray(attention_mask_l),
        v_proj_w=np.asarray(v_proj_w, np.float32), v_proj_b=np.asarray(v_proj_b, np.float32),
        l_proj_w=np.asarray(l_proj_w, np.float32), l_proj_b=np.asarray(l_proj_b, np.float32),
        values_v_w=np.asarray(values_v_w, np.float32), values_v_b=np.asarray(values_v_b, np.float32),
        values_l_w=np.asarray(values_l_w, np.float32), values_l_b=np.asarray(values_l_b, np.float32),
        out_v_w=np.asarray(out_v_w, np.float32), out_v_b=np.asarray(out_v_b, np.float32),
        out_l_w=np.asarray(out_l_w, np.float32), out_l_b=np.asarray(out_l_b, np.float32),
    )
    if (v.shape != (BSZ, NIMG, V_DIM) or l.shape != (BSZ, NTEXT, L_DIM)
            or args["attention_mask_v"].any() or args["attention_mask_l"].any()):
        return _numpy_fallback(v, l, **args)

    if "prog" not in _prog_cache:
        _prog_cache["prog"] = _build_program()
    nc = _prog_cache["prog"]

    shared = {
        "wq": np.ascontiguousarray(args["v_proj_w"].T) * np.float32(SCALE),
        "wl": np.ascontiguousarray(args["l_proj_w"].T),
        "wvv": np.ascontiguousarray(args["values_v_w"].T),
        "wvl": np.ascontiguousarray(args["values_l_w"].T),
        "wo": np.ascontiguousarray(args["out_v_w"].T),
        "bq": np.ascontiguousarray(
            (args["v_proj_b"] * SCALE).reshape(HEADS, 128).T),
        "bk": np.ascontiguousarray(args["l_proj_b"].reshape(HEADS, 128).T),
        "bvv": np.broadcast_to(args["values_v_b"], (128, EMBED)).copy(),
        "bvl": np.broadcast_to(args["values_l_b"], (128, EMBED)).copy(),
        "ident": np.eye(128, dtype=np.float32),
        "ones": np.ones((128, 128), np.float32),
    }
    in_maps = []
    for c in range(NCORES):
        b, half = c // 2, c % 2
        m = dict(shared)
        m["v"] = np.ascontiguousarray(v[b, half * HALF:(half + 1) * HALF])
        m["l"] = np.ascontiguousarray(l[b])
        in_maps.append(m)

    timed_iters = int(os.environ.get("BMHA_TIMED_ITERS", "0"))
    results = _run_spmd(nc, in_maps, timed_iters=timed_iters)

    out_v = np.empty((BSZ, NIMG, V_DIM), np.float32)
    out_l = np.empty((BSZ, NTEXT, L_DIM), np.float32)
    for b in range(BSZ):
        r0, r1 = results[2 * b], results[2 * b + 1]
        out_v[b, :HALF] = r0["out_v"]
        out_v[b, HALF:] = r1["out_v"]
        nl = (r0["nl"].astype(np.float64) + r1["nl"].astype(np.float64))
        zl = (r0["zl"].astype(np.float64) + r1["zl"].astype(np.float64))
        # nl[d, h*256 + j]; zl[p, h*2 + tlc] with j = tlc*128 + p
        N = nl.reshape(128, HEADS, NTEXT).transpose(1, 2, 0)      # [h, j, d]
        Z = zl.reshape(128, HEADS, NTLC).transpose(1, 2, 0).reshape(HEADS, NTEXT)
        pre = (N / Z[:, :, None]).transpose(1, 0, 2).reshape(NTEXT, EMBED)
        out_l[b] = (pre @ args["out_l_w"].T.astype(np.float64)
                    + args["out_l_b"]).astype(np.float32)
    if args["out_v_b"].any():
        out_v += args["out_v_b"]
    return out_v, out_l
